# revision 9
# baseline (speedup 1.0000x reference)
"""Multi-head cross-attention Trainium2 kernel (8-core SPMD, data-parallel).

Shards (batch=4) x (seq halves) across 8 NeuronCores; each core runs the
full q/kv/attention/out-proj pipeline for its 2048 query rows in bf16 with
fp32 PSUM accumulation.

Key tricks:
  - mask: reference adds +1.0 to logits of keys j < mask[b] before softmax.
    softmax(l + m) = e^m * e^l / sum  ->  fold e^m into V rows (and into the
    softmax-sum ones column), so masking costs nothing per tile.
  - softmax sums come from an extra ones column appended to V (head_dim 73);
    no vector reductions at all.
  - per-head K^T tiles are zero-padded to full 128-partition chunks so every
    matmul operand sits at base partition 0 (tile_position constraint).
  - normalization (1/sum) is applied after transposing per-head output to
    natural orientation, where rows are partitions and tensor_scalar works.
"""

import sys

sys.path.insert(0, "/opt/trn_rl_repo")

import ml_dtypes
import numpy as np

import concourse.bass as bass  # noqa: F401  (engine types via nc)
import concourse.mybir as mybir
import concourse.tile as tile
from concourse import bacc
from concourse.bass_utils import run_bass_kernel_spmd
from concourse.masks import make_identity

BF16 = mybir.dt.bfloat16
F32 = mybir.dt.float32
NPBF16 = ml_dtypes.bfloat16
AF = mybir.ActivationFunctionType

B, NSEQ, MKEY, D, H, DH = 4, 4096, 300, 1152, 16, 72
NCORES = 8
C = D // 128  # 9 feature chunks
KC = 3  # key chunks, keys padded 300 -> 384
MP = KC * 128
RG = 512  # query rows per group
SCALE = 1.0 / float(np.sqrt(DH))
ROWS_PER_CORE = B * NSEQ // NCORES  # 2048

LAST_EXEC_NS = None


def _head_segs(h):
    """Feature range [72h, 72h+72) of head h split at 128-chunk boundaries.

    Returns [(chunk, lo, hi)] with chunk-local partition range [lo, hi)."""
    f0, f1 = DH * h, DH * h + DH
    segs = []
    c = f0 // 128
    while c * 128 < f1:
        lo = max(f0, c * 128) - c * 128
        hi = min(f1, (c + 1) * 128) - c * 128
        segs.append((c, lo, hi))
        c += 1
    return segs


def _chunk_segs(c):
    """[(h, i, lo, hi)] head segments living in feature chunk c."""
    out = []
    for h in range(H):
        for i, (hc, lo, hi) in enumerate(_head_segs(h)):
            if hc == c:
                out.append((h, i, lo, hi))
    return out


# flat order of all (head, segment) pairs; column index into the hmask input
_ALL_SEGS = [(h, i) for h in range(H) for i in range(len(_head_segs(h)))]
_SEG_IDX = {hs: s for s, hs in enumerate(_ALL_SEGS)}
NSEG = len(_ALL_SEGS)


def _hmask_host():
    """[128, NSEG] f32: column (h,i) is 1.0 on the chunk-local partitions of
    that head segment, 0 elsewhere. Engine ops can't address SBUF at
    non-32-aligned partition bases, so head extraction is done as a
    full-chunk copy multiplied by this per-partition mask."""
    m = np.zeros((128, NSEG), np.float32)
    for h in range(H):
        for i, (_, lo, hi) in enumerate(_head_segs(h)):
            m[lo:hi, _SEG_IDX[(h, i)]] = 1.0
    return m


def build_program(rpc=ROWS_PER_CORE, has_bq=False, has_bk=False, has_bv=False, has_bp=False):
    nc = bacc.Bacc()

    xT_d = nc.dram_tensor("xT", [C, 128, rpc], BF16, kind="ExternalInput")
    condT_d = nc.dram_tensor("condT", [C, 128, MKEY], BF16, kind="ExternalInput")
    wq_d = nc.dram_tensor("wq", [C, 128, D], BF16, kind="ExternalInput")
    wk_d = nc.dram_tensor("wk", [C, 128, D], BF16, kind="ExternalInput")
    wv_d = nc.dram_tensor("wv", [C, 128, D], BF16, kind="ExternalInput")
    wp_d = nc.dram_tensor("wp", [C, 128, D], BF16, kind="ExternalInput")
    bq_d = nc.dram_tensor("bq", [128, C], F32, kind="ExternalInput")
    bk_d = nc.dram_tensor("bk", [128, C], F32, kind="ExternalInput")
    bv_d = nc.dram_tensor("bv", [1, D], BF16, kind="ExternalInput")
    bp_d = nc.dram_tensor("bp", [1, D], BF16, kind="ExternalInput")
    vs_d = nc.dram_tensor("vscale", [128, KC], F32, kind="ExternalInput")
    hm_d = nc.dram_tensor("hmask", [128, NSEG], F32, kind="ExternalInput")
    out_d = nc.dram_tensor("out", [rpc, D], F32, kind="ExternalOutput")

    groups = rpc // RG
    tiles_per_group = RG // 128
    kn = [128, 128, MKEY - 256]  # real keys per key chunk

    with tile.TileContext(nc) as tc:
        with tc.tile_pool(name="const", bufs=1) as cpool:
            # persistent weights / constants
            wq_sb = cpool.tile([128, C, D], BF16)
            wp_sb = cpool.tile([128, C, D], BF16)
            for k in range(C):
                nc.sync.dma_start(wq_sb[:, k, :], wq_d[k])
                nc.sync.dma_start(wp_sb[:, k, :], wp_d[k])
            ident = cpool.tile([128, 128], BF16)
            make_identity(nc, ident[:])
            vs_sb = cpool.tile([128, KC], F32)
            nc.sync.dma_start(vs_sb[:], vs_d[:])
            hm_sb = cpool.tile([128, NSEG], F32)
            nc.sync.dma_start(hm_sb[:], hm_d[:])
            if has_bq:
                bq_sb = cpool.tile([128, C], F32)
                nc.sync.dma_start(bq_sb[:], bq_d[:])
            if has_bk:
                bk_sb = cpool.tile([128, C], F32)
                nc.sync.dma_start(bk_sb[:], bk_d[:])
            if has_bp:
                bp_sb = cpool.tile([1, D], BF16)
                nc.sync.dma_start(bp_sb[:], bp_d[:])
            if has_bv or has_bp:
                ones_sb = cpool.tile([1, 128], BF16)
                nc.gpsimd.memset(ones_sb[:], 1.0)

            # V in natural orientation [key, head, dim+1]; fake keys stay 0,
            # col 72 holds e^mask (ones column pre-scaled by the mask factor)
            v_sb = cpool.tile([128, KC, H, DH + 1], BF16)
            nc.gpsimd.memset(v_sb[:], 0.0)
            kTz = {}
            for h in range(H):
                for i in range(len(_head_segs(h))):
                    t = cpool.tile([128, MP], BF16, name=f"kTz_{h}_{i}")
                    nc.gpsimd.memset(t[:], 0.0)
                    kTz[(h, i)] = t

            # ---- kv projection (weights scoped; SBUF/PSUM freed after) ----
            with (
                tc.tile_pool(name="kvw", bufs=1) as kvpool,
                tc.tile_pool(name="pskv", bufs=4, space="PSUM") as pskv,
            ):
                condT_sb = kvpool.tile([128, C, MKEY], BF16)
                wk_sb = kvpool.tile([128, C, D], BF16)
                wv_sb = kvpool.tile([128, C, D], BF16)
                for k in range(C):
                    nc.sync.dma_start(condT_sb[:, k, :], condT_d[k])
                    nc.sync.dma_start(wk_sb[:, k, :], wk_d[k])
                    nc.sync.dma_start(wv_sb[:, k, :], wv_d[k])
                if has_bv:
                    bv_sb = kvpool.tile([1, D], BF16)
                    nc.sync.dma_start(bv_sb[:], bv_d[:])

                # K^T in feature-chunk orientation -> zero-padded head tiles
                for c in range(C):
                    kps = pskv.tile([128, MKEY], F32, name="kps", tag="kv")
                    for k in range(C):
                        nc.tensor.matmul(
                            kps[:],
                            wk_sb[:, k, c * 128 : (c + 1) * 128],
                            condT_sb[:, k, :],
                            start=(k == 0),
                            stop=(k == C - 1),
                        )
                    for h, i, _lo, _hi in _chunk_segs(c):
                        s = _SEG_IDX[(h, i)]
                        if has_bk:
                            nc.vector.tensor_scalar(
                                kTz[(h, i)][:, 0:MKEY],
                                kps[:],
                                bk_sb[:, c : c + 1],
                                hm_sb[:, s : s + 1],
                                op0=mybir.AluOpType.add,
                                op1=mybir.AluOpType.mult,
                            )
                        else:
                            nc.vector.tensor_scalar_mul(
                                kTz[(h, i)][:, 0:MKEY], kps[:], hm_sb[:, s : s + 1]
                            )

                # V natural [keys, feat], head-aligned 360-wide chunks
                vch = [(0, 360), (360, 720), (720, 1080), (1080, 1152)]
                for kc in range(KC):
                    for f0, f1 in vch:
                        vps = pskv.tile([128, f1 - f0], F32, name="vps", tag="kv")
                        for k in range(C):
                            nc.tensor.matmul(
                                vps[0 : kn[kc], :],
                                condT_sb[:, k, kc * 128 : kc * 128 + kn[kc]],
                                wv_sb[:, k, f0:f1],
                                start=(k == 0),
                                stop=(k == C - 1 and not has_bv),
                            )
                        if has_bv:
                            nc.tensor.matmul(
                                vps[0 : kn[kc], :],
                                ones_sb[0:1, 0 : kn[kc]],
                                bv_sb[0:1, f0:f1],
                                start=False,
                                stop=True,
                            )
                        for h in range(f0 // DH, f1 // DH):
                            d0 = h * DH - f0
                            nc.vector.tensor_scalar_mul(
                                v_sb[0 : kn[kc], kc, h, 0:DH],
                                vps[0 : kn[kc], d0 : d0 + DH],
                                vs_sb[0 : kn[kc], kc : kc + 1],
                            )
                    for h in range(H):
                        nc.any.tensor_copy(
                            v_sb[0 : kn[kc], kc, h, DH : DH + 1],
                            vs_sb[0 : kn[kc], kc : kc + 1],
                        )

            # ---- streaming: q-proj, attention, out-proj ----
            with (
                tc.tile_pool(name="xq", bufs=2) as xqpool,
                tc.tile_pool(name="att", bufs=3) as apool,
                tc.tile_pool(name="outp", bufs=2) as opool,
                tc.tile_pool(name="psq", bufs=2, space="PSUM") as psq,
                tc.tile_pool(name="psa", bufs=6, space="PSUM") as psa,
            ):
                for g in range(groups):
                    xT_sb = xqpool.tile([128, C, RG], BF16, name="xT", tag="xT")
                    for k in range(C):
                        nc.sync.dma_start(xT_sb[:, k, :], xT_d[k][:, g * RG : (g + 1) * RG])
                    qT_sb = xqpool.tile([128, C, RG], BF16, name="qT", tag="qT")
                    for c in range(C):
                        qps = psq.tile([128, RG], F32, name="qps", tag="qps")
                        for k in range(C):
                            nc.tensor.matmul(
                                qps[:],
                                wq_sb[:, k, c * 128 : (c + 1) * 128],
                                xT_sb[:, k, :],
                                start=(k == 0),
                                stop=(k == C - 1),
                            )
                        if has_bq:
                            nc.scalar.activation(
                                qT_sb[:, c, :], qps[:], AF.Identity, bias=bq_sb[:, c : c + 1]
                            )
                        else:
                            nc.any.tensor_copy(qT_sb[:, c, :], qps[:])

                    for rt in range(tiles_per_group):
                        grt = g * tiles_per_group + rt
                        rs = slice(rt * 128, (rt + 1) * 128)
                        onat_sb = opool.tile([128, D], BF16, name="onat", tag="onat")
                        for h in range(H):
                            segs = _head_segs(h)
                            lps = psa.tile([128, KC, 128], F32, name="lps", tag="a")
                            for kc in range(KC):
                                for i, (c, lo, hi) in enumerate(segs):
                                    nc.tensor.matmul(
                                        lps[:, kc, :],
                                        kTz[(h, i)][:, kc * 128 : (kc + 1) * 128],
                                        qT_sb[:, c, rs],
                                        start=(i == 0),
                                        stop=(i == len(segs) - 1),
                                    )
                            expT = apool.tile([128, KC, 128], BF16, name="expT", tag="expT")
                            nc.scalar.activation(expT[:], lps[:], AF.Exp, scale=SCALE)
                            ops = psa.tile([DH + 1, 128], F32, name="ops", tag="a")
                            for kc in range(KC):
                                nc.tensor.matmul(
                                    ops[:],
                                    v_sb[:, kc, h, :],
                                    expT[:, kc, :],
                                    start=(kc == 0),
                                    stop=(kc == KC - 1),
                                )
                            oT_sb = apool.tile([DH + 1, 128], BF16, name="oT", tag="oT")
                            nc.any.tensor_copy(oT_sb[:], ops[:])
                            onp = psa.tile([128, DH + 1], BF16, name="onp", tag="a")
                            nc.tensor.transpose(onp[:], oT_sb[:], ident[0 : DH + 1, 0 : DH + 1])
                            inv = apool.tile([128, 1], F32, name="inv", tag="inv")
                            nc.vector.reciprocal(inv[:], onp[:, DH : DH + 1])
                            nc.vector.tensor_scalar_mul(
                                onat_sb[:, h * DH : (h + 1) * DH], onp[:, 0:DH], inv[:]
                            )

                        oTc_sb = opool.tile([128, C, 128], BF16, name="oTc", tag="oTc")
                        for c in range(C):
                            tps = psa.tile([128, 128], BF16, name="tps", tag="a")
                            nc.tensor.transpose(tps[:], onat_sb[:, c * 128 : (c + 1) * 128], ident[:])
                            nc.any.tensor_copy(oTc_sb[:, c, :], tps[:])

                        ysb = opool.tile([128, D], F32, name="ysb", tag="y")
                        for f0, f1 in [(0, 384), (384, 768), (768, 1152)]:
                            yps = psa.tile([128, f1 - f0], F32, name="yps", tag="a")
                            for c in range(C):
                                nc.tensor.matmul(
                                    yps[:],
                                    oTc_sb[:, c, :],
                                    wp_sb[:, c, f0:f1],
                                    start=(c == 0),
                                    stop=(c == C - 1 and not has_bp),
                                )
                            if has_bp:
                                nc.tensor.matmul(
                                    yps[:],
                                    ones_sb[0:1, :],
                                    bp_sb[0:1, f0:f1],
                                    start=False,
                                    stop=True,
                                )
                            nc.any.tensor_copy(ysb[:, f0:f1], yps[:])
                        nc.sync.dma_start(out_d[grt * 128 : (grt + 1) * 128, :], ysb[:])

    nc.compile()
    return nc


_programs = {}


def _get_program(key):
    if key not in _programs:
        _programs[key] = build_program(*key)
    return _programs[key]


def make_in_maps(x, cond, mask, Wq, bq, Wkv, bkv, Wp, bp, rpc=ROWS_PER_CORE, ncores=NCORES):
    """Host-side shard + relayout. Returns (in_maps, flags)."""
    x = np.asarray(x, np.float32)
    cond = np.asarray(cond, np.float32)
    mask = np.asarray(mask)
    Wq = np.asarray(Wq, np.float32)
    Wkv = np.asarray(Wkv, np.float32)
    Wp = np.asarray(Wp, np.float32)
    bq = np.asarray(bq, np.float32)
    bkv = np.asarray(bkv, np.float32)
    bp = np.asarray(bp, np.float32)

    wq = np.ascontiguousarray(Wq.astype(NPBF16).reshape(C, 128, D))
    wk = np.ascontiguousarray(Wkv[:, :D].astype(NPBF16).reshape(C, 128, D))
    wv = np.ascontiguousarray(Wkv[:, D:].astype(NPBF16).reshape(C, 128, D))
    wp = np.ascontiguousarray(Wp.astype(NPBF16).reshape(C, 128, D))
    bq_a = np.ascontiguousarray(bq.reshape(C, 128).T)
    bk_a = np.ascontiguousarray(bkv[:D].reshape(C, 128).T)
    bv_a = bkv[D:].astype(NPBF16).reshape(1, D)
    bp_a = bp.astype(NPBF16).reshape(1, D)

    flags = (rpc, bool(bq.any()), bool(bkv[:D].any()), bool(bkv[D:].any()), bool(bp.any()))
    hmask = _hmask_host()

    halves = NSEQ // rpc
    in_maps = []
    for core in range(ncores):
        b, half = core // halves, core % halves
        rows = slice(half * rpc, (half + 1) * rpc)
        xT = np.ascontiguousarray(x[b, rows].T.astype(NPBF16)).reshape(C, 128, rpc)
        condT = np.ascontiguousarray(cond[b].T.astype(NPBF16)).reshape(C, 128, MKEY)
        mv = (np.arange(MP) < int(mask[b])).astype(np.float32)
        vscale = np.ascontiguousarray(np.exp(mv).reshape(KC, 128).T)
        in_maps.append(
            {
                "xT": xT,
                "condT": condT,
                "wq": wq,
                "wk": wk,
                "wv": wv,
                "wp": wp,
                "bq": bq_a,
                "bk": bk_a,
                "bv": bv_a,
                "bp": bp_a,
                "vscale": vscale,
                "hmask": hmask,
            }
        )
    return in_maps, flags


def kernel(x, cond, mask, Wq, bq, Wkv, bkv, Wp, bp):
    global LAST_EXEC_NS
    import os

    in_maps, flags = make_in_maps(x, cond, mask, Wq, bq, Wkv, bkv, Wp, bp)
    nc = _get_program(flags)
    trace = bool(os.environ.get("BASS_KERNEL_TRACE"))
    res = run_bass_kernel_spmd(nc, in_maps, list(range(NCORES)), trace=trace)
    LAST_EXEC_NS = res.exec_time_ns

    rpc = flags[0]
    halves = NSEQ // rpc
    out = np.empty((B, NSEQ, D), np.float32)
    for core in range(NCORES):
        b, half = core // halves, core % halves
        out[b, half * rpc : (half + 1) * rpc] = res.results[core]["out"]
    return out
